# revision 15
# baseline (speedup 1.0000x reference)
"""GNN message-passing kernel (kapitza edge energies + segment_sum) on 8 TRN2 cores.

Strategy
--------
Shard by *target-node range*: core k owns nodes [k*125000, (k+1)*125000) and
receives every edge whose tgt falls in that range (host sorts edges by tgt).
Each core's output slice is independent -> no collective; host concatenates.

On each core the host lays edges out in a degree-padded CSR grid: each local
target node gets a row whose slot count is the smallest tier pad >= its
degree (tiers 4/6/8/10/12/16; degree>16 rows spill into a small overflow
grid).  That makes
  * T[tgt] a dense per-row broadcast (no gather),
  * the segment-sum a dense free-axis reduction (no scatter),
and leaves ONE indexed op: gathering the u64-packed {T, W}[src] pair
(8 B per edge) from an SBUF-resident node table.  W = 0.5*pi*L*D*avg_f is
precomputed densely on device (phase A).

The gather uses gpsimd indirect DMA with the empirically-validated HW
contract: a [1, K, 1] destination AP based at partition p yields K
per-element descriptors; descriptor j consumes index idx[j % 128, j // 128]
(column-major across the index tile's partitions); indices > bounds_check
are skipped leaving the destination untouched (pad slots).  One instruction
per partition (128 per core) gathers that partition's slots.  Instructions
round-robin over 4 SWDGE queues (n_queues=4): with a single queue the
queue FIFO serializes the per-instruction descriptor drain (~15.6 ms/core);
4 queues keep multiple SDMA rings busy (~1.7 ms/core measured).
"""

import sys
from contextlib import ExitStack
from dataclasses import dataclass

import numpy as np

if "/opt/trn_rl_repo" not in sys.path:
    sys.path.insert(0, "/opt/trn_rl_repo")

import concourse.bacc as bacc
import concourse.bass as bass
import concourse.tile as tile
from concourse import mybir
import concourse.mybir as mb
from concourse.tile_rust import add_dep_helper

F32 = mybir.dt.float32
I32 = mybir.dt.int32
U64 = mybir.dt.uint64
PI = float(np.pi)


@dataclass(frozen=True)
class Cfg:
    N_NODES: int  # real node count
    NT_PART: int  # node-table cols per partition (NT_PAD = 128*NT_PART)
    NT_CHUNKS: int  # phase-A chunks
    NODES_PC: int  # real nodes per core
    tiers: tuple  # ((pad, rows_per_partition), ...) ascending pads
    OV_ROWS: int  # overflow rows (spread over OV_PARTS partitions)
    OV_PAD: int  # overflow slots per row
    OV_PARTS: int = 16
    n_queues: int = 4  # SWDGE queues for gather instructions
    qmap: str = "mod"  # 'mod': q=p%nq ; 'port': q=sdma_port(p)%nq
    table_sbuf: bool = False  # node table resident in SBUF (encoded indices)
    pair_units: bool = False  # 16B descriptors: 2 slots/desc, paired table
    sim_order: bool = False  # True: interp's row-major idx order (CoreSim)
    loop_reps: int = 0  # timing variant: repeat gather+compute in a For_i
    debug: bool = False

    @property
    def NT_PAD(self):
        return 128 * self.NT_PART

    @property
    def NT_CR(self):
        return self.NT_PART // self.NT_CHUNKS

    @property
    def SLOTS_P(self):
        return sum(pad * rpp for pad, rpp in self.tiers)  # slots per partition

    @property
    def R_PART_EFF(self):
        return sum(rpp for _, rpp in self.tiers)

    @property
    def MAXPAD(self):
        return self.tiers[-1][0]

    @property
    def N_UNITS(self):
        # descriptors per partition-instruction (2 slots/desc in pair mode)
        return self.SLOTS_P // (2 if self.pair_units else 1)

    @property
    def IDX_C(self):
        return self.N_UNITS // 128  # idx-tile columns per partition-instruction

    @property
    def CHUNK(self):
        return 1024 if self.pair_units else 2048  # compute-chunk slots (SBUF fit)

    @property
    def OV_S(self):
        return self.OV_ROWS * self.OV_PAD  # total overflow slots

    @property
    def OV_RPP(self):
        return self.OV_ROWS // self.OV_PARTS  # overflow rows per used partition

    @property
    def OV_SPP(self):
        return self.OV_S // self.OV_PARTS  # overflow slots per used partition


FULL = Cfg(
    N_NODES=1_000_000,
    NT_PART=7816,  # NT_PAD = 1_000_448
    NT_CHUNKS=8,
    NODES_PC=125_000,
    # rows-per-partition sized for the observed per-core degree histogram
    # (max rows needed across cores: [12549, 27061, 35140, 28097, 15233, 8046]
    #  -> capacities [14208, 29440, 38016, 30464, 16512, 8704], ~8% slack;
    #  host_prep cascades nodes into higher tiers if a tier fills up).
    tiers=((4, 111), (6, 230), (8, 297), (10, 238), (12, 129), (16, 68)),
    OV_ROWS=1024,  # measured: <= 504 used per core
    OV_PAD=12,  # supports degree <= 16 + 12 = 28 (measured max 24)
    n_queues=4,
    qmap="port",  # spread concurrent instructions across SDMA ports
    table_sbuf=True,
)

FULL_P = Cfg(
    N_NODES=1_000_000,
    NT_PART=7816,
    NT_CHUNKS=8,
    NODES_PC=125_000,
    # pair mode: tiers are SLOT pads = 2 * unit pads; each 16B descriptor
    # covers 2 adjacent slots.  Unit pads 1..16 sized from the per-core
    # (deg<=16 minus matched-pairs) histogram, worst core, +8% slack.
    tiers=((2, 45), (4, 40), (6, 102), (8, 173), (10, 212), (12, 200),
           (14, 151), (16, 95), (20, 76), (24, 14), (32, 1)),
    OV_ROWS=1024,
    OV_PAD=12,
    n_queues=4,
    qmap="port",
    table_sbuf=True,
    pair_units=True,
)

N_CORES = 8


PSTRIDE64 = 256 * 1024 // 8  # SBUF partition stride in u64 elements (cayman)


def _sdma_port(p):
    """SDMA engine/port serving partition p (descriptor swizzle)."""
    return ((p % 32) // 4) * 2 + p // 64


def _sentinel(cfg):
    if cfg.table_sbuf:
        return np.int32(1 << 23)
    return np.int32(cfg.N_NODES)  # > bounds_check = N_NODES-1 -> skipped


def _encode_idx(cfg, n):
    """Node id -> gather index (SBUF mode encodes the partition stride)."""
    if not cfg.table_sbuf:
        return n.astype(np.int32)
    return ((n // cfg.NT_PART) * PSTRIDE64 + (n % cfg.NT_PART)).astype(np.int32)


# --------------------------------------------------------------------------
# device program
# --------------------------------------------------------------------------
def build_program(cfg: Cfg):
    assert cfg.SLOTS_P % 128 == 0 and cfg.OV_S % (128 * cfg.OV_PARTS) == 0
    assert cfg.OV_ROWS % 128 == 0
    nc = bacc.Bacc(
        "TRN2", target_bir_lowering=False, debug=cfg.debug,
        num_swdge_queues=max(cfg.n_queues, 1),
    )

    Tf = nc.dram_tensor("T_full", [128, cfg.NT_PART], F32, kind="ExternalInput")
    Lf = nc.dram_tensor("L_full", [128, cfg.NT_PART], F32, kind="ExternalInput")
    Df = nc.dram_tensor("D_full", [128, cfg.NT_PART], F32, kind="ExternalInput")
    Ff = nc.dram_tensor("F_full", [128, cfg.NT_PART], F32, kind="ExternalInput")
    Tloc = nc.dram_tensor("T_loc", [128, cfg.R_PART_EFF], F32, kind="ExternalInput")
    ts = nc.dram_tensor("ts", [128, 1], F32, kind="ExternalInput")
    cond1 = nc.dram_tensor("cond1", [128, cfg.SLOTS_P], F32, kind="ExternalInput")
    # per-partition-instruction index tiles: cols [p*IDX_C, (p+1)*IDX_C)
    src1t = nc.dram_tensor("src1t", [128, cfg.N_UNITS], I32, kind="ExternalInput")
    ovc = nc.dram_tensor("ov_cond", [cfg.OV_PARTS, cfg.OV_SPP], F32, kind="ExternalInput")
    ovst = nc.dram_tensor("ov_srct", [128, cfg.OV_S // 128], I32, kind="ExternalInput")
    ovnt = nc.dram_tensor("ov_nodet", [128, cfg.OV_ROWS // 128], I32, kind="ExternalInput")
    ovot = nc.dram_tensor("ov_outt", [128, cfg.OV_ROWS // 128], I32, kind="ExternalInput")
    out = nc.dram_tensor("out", [128, cfg.R_PART_EFF], F32, kind="ExternalOutput")

    # u64-packed {T, W} node table (DRAM fallback when table_sbuf=False)
    table = nc.dram_tensor("tableTW", [cfg.NT_PAD, 2], F32)
    table_w = table[:].rearrange("(p f) two -> p (f two)", p=128)
    table64 = table[:].bitcast(U64)  # [NT_PAD, 1]

    def q_of(i):
        nq = max(cfg.n_queues, 1)
        qi = (_sdma_port(i) if cfg.qmap == "port" else i) % nq
        return f"qPoolDynamic{qi or ''}"

    sbuf_src = {"ap": None}  # set to the SBUF table AP in table_sbuf mode

    def gather64(out_ap_1k1, idx_ap, deps, qname, pair=False):
        if not cfg.table_sbuf:
            assert not pair
            g = nc.gpsimd.indirect_dma_start(
                out=out_ap_1k1,
                out_offset=None,
                in_=table64,
                in_offset=bass.IndirectOffsetOnAxis(ap=idx_ap, axis=0),
                bounds_check=cfg.N_NODES - 1,
                oob_is_err=False,
            )
            g.ins.queue = qname
        else:
            gp = nc.gpsimd
            src_ap = sbuf_src["ap"]
            if pair:
                src_ap = src_ap.rearrange("p (h two) -> p h two", two=2)
            out_l = gp.lower_ap_dma(out_ap_1k1, for_indirect_dma=True)
            in_l = gp.lower_ap_dma(src_ap, for_indirect_dma=True)
            off_l = gp.lower_ap_dma(idx_ap)
            assert len(out_l) == 1 and len(in_l) == 1 and len(off_l) == 1
            in_l.append(off_l[0])
            in_l[0].dynamic_ap_info = mb.DynamicAccessPatternInfo(
                c=0,
                actual_ap=out_l[0].ap,
                indirect_dim_max_index=cfg.NT_PAD,
                offset_expr=[
                    mb.DynamicAccessPatternOffsetExpr(
                        coef=2 if pair else 1,
                        aff_expr=mb.DynamicAccessPatternOffsetExprAffExpr(
                            kind="IndirectArgId", arg_id=1
                        ),
                    )
                ],
            )
            bound = 127 * PSTRIDE64 + cfg.NT_PART
            if pair:
                bound //= 2
            bc = [gp.lower_val_access(gp.to_reg(int(bound)))]
            g = gp.add_instruction(
                mb.InstDMACopy(
                    name=nc.get_next_instruction_name(),
                    queue=qname,
                    mode="Copy",
                    ins=in_l + bc,
                    outs=out_l,
                    oob_is_err=False,
                    cce_op=mb.AluOpType.bypass,
                )
            )
        for w in deps:
            add_dep_helper(g.ins, w.ins, reason="table RAW")
        return g

    with tile.TileContext(nc) as tc, ExitStack() as ctx:
        pts = ctx.enter_context(tc.tile_pool(name="pts", bufs=1))
        tts = pts.tile([128, 1], F32)
        nc.sync.dma_start(tts[:], ts[:])
        ttab = None
        if cfg.table_sbuf:
            ttab = pts.tile([128, 2 * cfg.NT_PART], F32)
            sbuf_src["ap"] = ttab[0:1, :].bitcast(U64)

        # ---- phase A: W = 0.5*pi*L*D*avg_f; write packed {T,W} table ----
        pa_ctx = ExitStack()
        pa = pa_ctx.enter_context(tc.tile_pool(name="pa", bufs=2))
        table_writes = []
        for j in range(cfg.NT_CHUNKS):
            sl = slice(j * cfg.NT_CR, (j + 1) * cfg.NT_CR)
            tT = pa.tile([128, cfg.NT_CR], F32, tag="tT")
            nc.sync.dma_start(tT[:], Tf[:, sl])
            tL = pa.tile([128, cfg.NT_CR], F32, tag="tL")
            nc.sync.dma_start(tL[:], Lf[:, sl])
            tD = pa.tile([128, cfg.NT_CR], F32, tag="tD")
            nc.sync.dma_start(tD[:], Df[:, sl])
            tF = pa.tile([128, cfg.NT_CR], F32, tag="tF")
            nc.sync.dma_start(tF[:], Ff[:, sl])
            tW = pa.tile([128, cfg.NT_CR], F32, tag="tW")
            nc.vector.tensor_mul(tW[:], tL[:], tD[:])
            nc.vector.tensor_mul(tW[:], tW[:], tF[:])
            nc.vector.tensor_scalar_mul(tW[:], tW[:], 0.5 * PI)
            if cfg.table_sbuf:
                dst = ttab[:, j * cfg.NT_CR * 2 : (j + 1) * cfg.NT_CR * 2]
                ev = dst.rearrange("p (f two) -> p f two", two=2)
                w1 = nc.vector.tensor_copy(ev[:, :, 0:1], tT[:].unsqueeze(2))
                w2 = nc.vector.tensor_copy(ev[:, :, 1:2], tW[:].unsqueeze(2))
                table_writes.extend([w1, w2])
            else:
                tTW = pa.tile([128, cfg.NT_CR * 2], F32, tag="tTW")
                ev = tTW[:].rearrange("p (f two) -> p f two", two=2)
                nc.vector.tensor_copy(ev[:, :, 0:1], tT[:].unsqueeze(2))
                nc.vector.tensor_copy(ev[:, :, 1:2], tW[:].unsqueeze(2))
                w = nc.sync.dma_start(
                    table_w[:, j * cfg.NT_CR * 2 : (j + 1) * cfg.NT_CR * 2], tTW[:]
                )
                table_writes.append(w)
        pa_ctx.close()

        # ---- phase B: local T, T^3 * time_step ----
        main_ctx = ExitStack()
        pers = main_ctx.enter_context(tc.tile_pool(name="pers", bufs=1))
        tTloc = pers.tile([128, cfg.R_PART_EFF], F32)
        nc.sync.dma_start(tTloc[:], Tloc[:])
        tT3 = pers.tile([128, cfg.R_PART_EFF], F32)
        nc.vector.tensor_mul(tT3[:], tTloc[:], tTloc[:])
        nc.vector.tensor_mul(tT3[:], tT3[:], tTloc[:])
        nc.vector.tensor_scalar(
            tT3[:], tT3[:], tts[:, 0:1], None, op0=mybir.AluOpType.mult
        )

        # ---- gather + compute, single sweep ----
        ptw = main_ctx.enter_context(tc.tile_pool(name="ptw", bufs=1))
        pidx = main_ctx.enter_context(tc.tile_pool(name="pidx", bufs=4))
        pc = main_ctx.enter_context(tc.tile_pool(name="pc", bufs=2))
        loop_ctx = ExitStack()
        if cfg.loop_reps > 0:
            loop_ctx.enter_context(tc.For_i(0, cfg.loop_reps, 1))
        # chunk plan: (pad, row0, nrows, slot0) within tier regions
        regions = []
        ro = so = 0
        for pad, rpp in cfg.tiers:
            regions.append((pad, rpp, ro, so))
            ro += rpp
            so += pad * rpp
        chunk_plan = []
        for pad, rpp, ro, so in regions:
            s = so
            while s < so + pad * rpp:
                e = min(s + cfg.CHUNK, so + pad * rpp)
                e = s + ((e - s) // pad) * pad
                chunk_plan.append((pad, ro + (s - so) // pad, (e - s) // pad, s))
                s = e
        out_writes = []
        TW = ptw.tile([128, 2 * cfg.SLOTS_P], F32, tag="tw")
        nc.vector.memset(TW[:], 0.0)
        for p in range(128):
            ti = pidx.tile([128, cfg.IDX_C], I32, tag="idx")
            nc.sync.dma_start(
                ti[:], src1t[:, p * cfg.IDX_C : (p + 1) * cfg.IDX_C]
            )
            tw64 = TW[p : p + 1, :].bitcast(U64)
            out_ap = (
                tw64.rearrange("p (k l) -> p k l", l=2)
                if cfg.pair_units
                else tw64.unsqueeze(2)
            )
            gather64(out_ap, ti[:], table_writes, q_of(p), pair=cfg.pair_units)

        for pad, r0, nr, s0 in chunk_plan:
            cs = nr * pad
            tcond = pc.tile([128, cfg.CHUNK], F32, tag="cond")
            nc.sync.dma_start(tcond[:, :cs], cond1[:, s0 : s0 + cs])
            tw4 = TW[:, 2 * s0 : 2 * (s0 + cs)].rearrange(
                "p (r s two) -> p r s two", s=pad, two=2
            )
            Ts = tw4[:, :, :, 0:1].squeeze(3)
            Ws = tw4[:, :, :, 1:2].squeeze(3)
            Tt = tTloc[:, r0 : r0 + nr].unsqueeze(2).broadcast_to([128, nr, pad])
            T3 = tT3[:, r0 : r0 + nr].unsqueeze(2).broadcast_to([128, nr, pad])
            te = pc.tile([128, cfg.CHUNK], F32, tag="e")
            te3 = te[:, :cs].rearrange("p (r s) -> p r s", s=pad)
            nc.vector.tensor_tensor(te3, Ts, Tt, op=mybir.AluOpType.subtract)
            nc.vector.tensor_scalar_max(te[:, :cs], te[:, :cs], 0.0)
            nc.vector.tensor_tensor(te3, te3, Ws, op=mybir.AluOpType.mult)
            nc.vector.tensor_mul(te[:, :cs], te[:, :cs], tcond[:, :cs])
            nc.vector.tensor_tensor(te3, te3, T3, op=mybir.AluOpType.mult)
            tred = pc.tile([128, 512], F32, tag="red")
            nc.vector.tensor_reduce(
                tred[:, :nr], te3, axis=mybir.AxisListType.X, op=mybir.AluOpType.add
            )
            w = nc.sync.dma_start(out[:, r0 : r0 + nr], tred[:, :nr])
            out_writes.append(w)

        loop_ctx.close()
        main_ctx.close()
        if cfg.loop_reps == 0:
            # ---- phase D: overflow rows (degree > MAXPAD) on OV_PARTS partitions ----
            po = ctx.enter_context(tc.tile_pool(name="po", bufs=1))
            pidx = ctx.enter_context(tc.tile_pool(name="pidx2", bufs=2))
            NP, SPP, RPP = cfg.OV_PARTS, cfg.OV_SPP, cfg.OV_RPP
            toc = po.tile([128, SPP], F32)
            nc.sync.dma_start(toc[:NP, :], ovc[:])
            # slot pairs: NP instructions, one per used partition
            toTW = po.tile([128, 2 * SPP], F32)
            nc.vector.memset(toTW[:], 0.0)
            ovs_cols = cfg.OV_S // 128 // NP  # idx cols per partition-instruction
            for q in range(NP):
                ti = pidx.tile([128, ovs_cols], I32, tag="ovidx")
                nc.sync.dma_start(ti[:], ovst[:, q * ovs_cols : (q + 1) * ovs_cols])
                gather64(
                    toTW[q : q + 1, :].bitcast(U64).unsqueeze(2), ti[:], table_writes,
                    q_of(q),
                )
            # target-T pairs for all OV_ROWS via one instruction into partition 0
            tno = pidx.tile([128, cfg.OV_ROWS // 128], I32, tag="ovn")
            nc.sync.dma_start(tno[:], ovnt[:])
            toTt0 = po.tile([128, 2 * cfg.OV_ROWS], F32)  # partition 0 row used
            nc.vector.memset(toTt0[0:1, :], 0.0)
            gather64(
                toTt0[0:1, :].bitcast(U64).unsqueeze(2), tno[:], table_writes, q_of(0)
            )
            # spread partition-0 pair stream -> [NP, 2*RPP]
            toTt = po.tile([128, 2 * RPP], F32)
            nc.sync.dma_start(toTt[:NP, :], toTt0[0:1, :])

            ttpair = toTt[:NP, :].rearrange("p (r two) -> p r two", two=2)
            tt_col = ttpair[:, :, 0:1]  # [NP, RPP, 1] target T
            to3 = po.tile([128, RPP], F32)
            nc.vector.tensor_tensor(
                to3[:NP, :].unsqueeze(2), tt_col, tt_col, op=mybir.AluOpType.mult
            )
            nc.vector.tensor_tensor(
                to3[:NP, :].unsqueeze(2), to3[:NP, :].unsqueeze(2), tt_col,
                op=mybir.AluOpType.mult,
            )
            nc.vector.tensor_scalar(
                to3[:NP, :], to3[:NP, :], tts[:NP, 0:1], None, op0=mybir.AluOpType.mult
            )

            ow4 = toTW[:NP, :].rearrange("p (r s two) -> p r s two", s=cfg.OV_PAD, two=2)
            oTs = ow4[:, :, :, 0:1].squeeze(3)
            oWs = ow4[:, :, :, 1:2].squeeze(3)
            oTt = tt_col.broadcast_to([NP, RPP, cfg.OV_PAD])
            oT3 = to3[:NP, :].unsqueeze(2).broadcast_to([NP, RPP, cfg.OV_PAD])
            teo = po.tile([128, SPP], F32)
            teo3 = teo[:NP, :].rearrange("p (r s) -> p r s", s=cfg.OV_PAD)
            nc.vector.tensor_tensor(teo3, oTs, oTt, op=mybir.AluOpType.subtract)
            nc.vector.tensor_scalar_max(teo[:NP, :], teo[:NP, :], 0.0)
            nc.vector.tensor_tensor(teo3, teo3, oWs, op=mybir.AluOpType.mult)
            nc.vector.tensor_mul(teo[:NP, :], teo[:NP, :], toc[:NP, :])
            nc.vector.tensor_tensor(teo3, teo3, oT3, op=mybir.AluOpType.mult)
            tosum = po.tile([128, RPP], F32)
            nc.vector.tensor_reduce(
                tosum[:NP, :], teo3, axis=mybir.AxisListType.X, op=mybir.AluOpType.add
            )
            # collapse [NP, RPP] -> partition-0 stream [1, OV_ROWS]
            tosum0 = po.tile([128, cfg.OV_ROWS], F32)
            nc.sync.dma_start(tosum0[0:1, :], tosum[:NP, :])

            # gather current out rows, add, scatter back (f32 elements)
            too = pidx.tile([128, cfg.OV_ROWS // 128], I32, tag="ovo")
            nc.sync.dma_start(too[:], ovot[:])
            out_flat = out[:].rearrange("p r -> (p r)").unsqueeze(1)
            tcur = po.tile([128, cfg.OV_ROWS], F32)
            g3 = nc.gpsimd.indirect_dma_start(
                out=tcur[0:1, :].unsqueeze(2),
                out_offset=None,
                in_=out_flat,
                in_offset=bass.IndirectOffsetOnAxis(ap=too[:], axis=0),
                bounds_check=None,
                oob_is_err=False,
            )
            for w in out_writes:
                add_dep_helper(g3.ins, w.ins, reason="out RAW before ov add")
            nc.vector.tensor_add(tosum0[0:1, :], tosum0[0:1, :], tcur[0:1, :])
            sc = nc.gpsimd.indirect_dma_start(
                out=out_flat,
                out_offset=bass.IndirectOffsetOnAxis(ap=too[:], axis=0),
                in_=tosum0[0:1, :].unsqueeze(2),
                in_offset=None,
                bounds_check=None,
                oob_is_err=False,
            )
            for w in out_writes:
                add_dep_helper(sc.ins, w.ins, reason="out WAW after level-1")
            add_dep_helper(sc.ins, g3.ins, reason="ov scatter after gather")

    nc.compile()
    return nc


# --------------------------------------------------------------------------
# host-side sharding / layout
# --------------------------------------------------------------------------
def _wrap_cols(cfg: Cfg, flat, cols):
    """Lay a flat stream so HW consumes it in order: idx[j%128, j//128]=flat[j].

    With sim_order (interp semantics) consumption is row-major over [128, C]."""
    if cfg.sim_order:
        return flat.reshape(128, cols)
    return np.ascontiguousarray(flat.reshape(cols, 128).T)


_ROWMAPS = {}


def _greedy_match(a, b, w, N):
    """Parallel greedy matching on the weighted pair graph (by -w order)."""
    partner = np.full(N, -1, np.int64)
    o = np.argsort(-w, kind="stable")
    a, b = a[o], b[o]
    alive = np.ones(len(a), bool)
    for _ in range(200):
        if not alive.any():
            break
        ai, bi = a[alive], b[alive]
        pos = np.arange(len(ai))
        first = np.full(N, 1 << 60, np.int64)
        np.minimum.at(first, ai, pos)
        np.minimum.at(first, bi, pos)
        acc = (first[ai] == pos) & (first[bi] == pos) & (ai != bi)
        x, y = ai[acc], bi[acc]
        partner[x] = y
        partner[y] = x
        idx = np.flatnonzero(alive)
        dead = (partner[a[idx]] >= 0) | (partner[b[idx]] >= 0)
        alive[idx[dead]] = False
    return partner


def _grouped_rank(keys):
    """rank of each element within its equal-key group (keys arbitrary)."""
    o = np.argsort(keys, kind="stable")
    ks = keys[o]
    new = np.concatenate([[True], ks[1:] != ks[:-1]])
    firstpos = np.maximum.accumulate(np.where(new, np.arange(len(ks)), 0))
    r = np.empty(len(keys), np.int64)
    r[o] = np.arange(len(ks)) - firstpos
    return r


def host_prep_pairs(cfg: Cfg, T, L, D, avg_f, conductivity, src, tgt, time_step):
    """Pair-descriptor layout: per-core permuted node table; each 16B
    descriptor covers 2 adjacent slots (a matched src pair, or 1 real edge +
    1 sacrificial cond=0 slot).  Units at stream position j==0 (mod 32), j>0
    get lane 0 corrupted by HW (stale finite reads), as do sentinel units —
    the layout parks only cond=0 slots there."""
    N = cfg.N_NODES
    T = np.asarray(T, np.float32)
    L = np.asarray(L, np.float32)
    D = np.asarray(D, np.float32)
    avg_f = np.asarray(avg_f, np.float32)
    cond = np.asarray(conductivity, np.float32)
    src = np.asarray(src, np.int32)
    tgt = np.asarray(tgt, np.int32)
    ts = np.full((128, 1), np.float32(np.asarray(time_step)), np.float32)
    SENT = _sentinel(cfg)
    MAXPAD = cfg.MAXPAD // 2  # unit pads

    def enc1(pos):
        return ((pos // cfg.NT_PART) * PSTRIDE64 + pos % cfg.NT_PART).astype(np.int32)

    def enc2(pos):  # pair-unit encoding (pos may be odd; unit = pos//2)
        return (
            (pos // cfg.NT_PART) * (PSTRIDE64 // 2) + (pos % cfg.NT_PART) // 2
        ).astype(np.int32)

    order = np.argsort(tgt, kind="stable")
    tgt_s = tgt[order]
    src_s = src[order]
    cond_s = cond[order]

    upads = np.array([p // 2 for p, _ in cfg.tiers], np.int64)
    rpps = np.array([r for _, r in cfg.tiers], np.int64)
    caps = rpps * 128
    row_off = np.concatenate([[0], np.cumsum(rpps)])
    unit_off = np.concatenate([[0], np.cumsum(upads * rpps)])
    NC = len(cfg.tiers)
    REFF = cfg.R_PART_EFF
    UNITS_P = cfg.N_UNITS

    in_maps = []
    for k in range(N_CORES):
        base = k * cfg.NODES_PC
        lo, hi = np.searchsorted(tgt_s, [base, base + cfg.NODES_PC])
        n = (tgt_s[lo:hi] - base).astype(np.int64)
        s = src_s[lo:hi].astype(np.int64)
        c = cond_s[lo:hi]
        deg = np.bincount(n, minlength=cfg.NODES_PC)
        starts = np.concatenate([[0], np.cumsum(deg)[:-1]])
        rank = np.arange(len(n), dtype=np.int64) - starts[n]
        main = rank < MAXPAD
        n2, s2, c2 = n[main], s[main], c[main]

        # ---- match srcs that co-occur in rows; matched pair -> adjacent table slots
        cands = []
        for d1 in range(1, MAXPAD):
            i = np.arange(len(s2) - d1)
            same = n2[i] == n2[i + d1]
            cands.append(
                np.stack(
                    [np.minimum(s2[i[same]], s2[i + d1][same]),
                     np.maximum(s2[i[same]], s2[i + d1][same])], 1
                )
            )
        P = np.concatenate(cands)
        uk, cnt = np.unique(P[:, 0] * N + P[:, 1], return_counts=True)
        partner = _greedy_match(uk // N, uk % N, cnt, N)

        # ---- permutation: pair (x<y) at positions (2i, 2i+1)
        xs = np.flatnonzero((partner >= 0) & (partner > np.arange(N)))
        ys = partner[xs]
        node_at_pos = np.zeros(cfg.NT_PAD, np.int64)
        node_at_pos[0 : 2 * len(xs) : 2] = xs
        node_at_pos[1 : 2 * len(xs) : 2] = ys
        un = np.flatnonzero(partner < 0)
        node_at_pos[2 * len(xs) : 2 * len(xs) + len(un)] = un
        pos_of = np.empty(N, np.int64)
        pos_of[node_at_pos[:N]] = np.arange(N)

        def padded_perm(x):
            return np.ascontiguousarray(
                x[node_at_pos].astype(np.float32).reshape(128, cfg.NT_PART)
            )

        # ---- per-row pair formation (k-th x-side edge with k-th y-side edge)
        pid_of = np.where(partner >= 0, np.minimum(np.arange(N), partner), -1)
        epid = pid_of[s2]
        valid = epid >= 0
        is_x = s2 == epid
        ckey = n2 * (2 * N) + np.where(valid, epid, 0)
        grank = _grouped_rank(ckey * 2 + is_x)
        kx = ckey[valid & is_x]
        ky = ckey[valid & ~is_x]
        ukx, cx = np.unique(kx, return_counts=True)
        uky, cy = np.unique(ky, return_counts=True)
        common, ix, iy = np.intersect1d(ukx, uky, return_indices=True)
        minc = np.minimum(cx[ix], cy[iy])
        idxc = np.minimum(np.searchsorted(common, ckey), max(len(common) - 1, 0))
        found = (len(common) > 0) & (common[idxc] == ckey)
        formed = valid & found & (grank < minc[idxc])
        fx = formed & is_x
        fy = formed & ~is_x
        ex = np.flatnonzero(fx)[np.argsort(ckey[fx] * 32 + grank[fx], kind="stable")]
        ey = np.flatnonzero(fy)[np.argsort(ckey[fy] * 32 + grank[fy], kind="stable")]
        assert len(ex) == len(ey) and (n2[ex] == n2[ey]).all()
        prow = n2[ex]
        punit = enc2(pos_of[s2[ex]])
        pc0, pc1 = c2[ex], c2[ey]
        se = np.flatnonzero(~formed)
        srow = n2[se]
        spos = pos_of[s2[se]]
        sunit = enc2(spos)
        slane = (spos % 2).astype(np.int64)
        sc = c2[se]

        np_r = np.bincount(prow, minlength=cfg.NODES_PC)
        ns_r = np.bincount(srow, minlength=cfg.NODES_PC)
        u_r = np_r + ns_r
        odd_r = np.bincount(srow[slane == 1], minlength=cfg.NODES_PC)

        # ---- class assignment (unit tiers) with cascade
        cls = np.searchsorted(upads, np.maximum(u_r, 1))
        for ci in range(NC - 1):
            idx = np.flatnonzero(cls == ci)
            if len(idx) > caps[ci]:
                cls[idx[caps[ci] :]] = ci + 1
        assert (cls == NC - 1).sum() <= caps[NC - 1], "top tier overflow"

        # ---- placement: corrupt-containing row positions get safe rows
        rowp = np.empty(cfg.NODES_PC, np.int64)
        rowr = np.empty(cfg.NODES_PC, np.int64)
        ustart = np.empty(cfg.NODES_PC, np.int64)
        has_cor = np.zeros(cfg.NODES_PC, bool)
        t_of = np.zeros(cfg.NODES_PC, np.int64)
        for ci in range(NC):
            nodes_c = np.flatnonzero(cls == ci)
            Pu = upads[ci]
            nq = len(nodes_c)
            q = np.arange(caps[ci])
            st = unit_off[ci] + (q // 128) * Pu
            m32 = ((np.maximum(st, 1) + 31) // 32) * 32
            cor = m32 < st + Pu
            safe = (u_r[nodes_c] < Pu) | (odd_r[nodes_c] > 0)
            qn = np.flatnonzero(~cor)
            qc = np.flatnonzero(cor)
            nu = int((~safe).sum())
            assert nu <= len(qn), (ci, nu, len(qn))
            order_n = np.concatenate([nodes_c[~safe], nodes_c[safe]])
            order_q = np.concatenate([qn[:nu], qc, qn[nu:]])[:nq]
            rowp[order_n] = order_q % 128
            rowr[order_n] = row_off[ci] + order_q // 128
            ustart[order_n] = st[order_q]
            has_cor[order_n] = cor[order_q]
            t_of[order_n] = m32[order_q] - st[order_q]

        # ---- fill unit/cond arrays
        unit_idx = np.full(128 * UNITS_P, SENT, np.int32)
        condA = np.zeros(128 * 2 * UNITS_P, np.float32)
        prank = _grouped_rank(prow)
        pu_pos = rowp[prow] * UNITS_P + ustart[prow] + prank
        unit_idx[pu_pos] = punit
        condA[2 * pu_pos] = pc0
        condA[2 * pu_pos + 1] = pc1
        srank = _grouped_rank(srow)
        su_pos = rowp[srow] * UNITS_P + ustart[srow] + np_r[srow] + srank
        unit_idx[su_pos] = sunit
        condA[2 * su_pos + slane] = sc

        # ---- corrupt fixups: unit at t must have cond lane0 == 0
        condV = condA.reshape(-1, 2)
        for node in np.flatnonzero(has_cor & (u_r > 0)):
            bu = rowp[node] * UNITS_P + ustart[node]
            t = int(t_of[node])
            if condV[bu + t, 0] == 0.0:
                continue
            Pu = int(upads[cls[node]])
            row_c0 = condV[bu : bu + Pu, 0]
            cand = np.flatnonzero(row_c0 == 0.0)
            assert len(cand), (node, Pu)
            t2 = int(cand[0])
            unit_idx[[bu + t, bu + t2]] = unit_idx[[bu + t2, bu + t]]
            condV[[bu + t, bu + t2]] = condV[[bu + t2, bu + t]]

        rowmap = rowp * REFF + rowr
        _ROWMAPS[k] = rowmap
        unit_p = unit_idx.reshape(128, UNITS_P)
        src1t = np.concatenate(
            [_wrap_cols(cfg, unit_p[p], cfg.IDX_C) for p in range(128)], axis=1
        )

        # ---- overflow grid (deg > MAXPAD), single-element descriptors
        m2 = ~main
        ov_nodes = np.unique(n[m2])
        assert len(ov_nodes) <= cfg.OV_ROWS
        assert deg.max() <= MAXPAD + cfg.OV_PAD, deg.max()
        ov_row_of = np.full(cfg.NODES_PC, -1, np.int64)
        ov_row_of[ov_nodes] = np.arange(len(ov_nodes))
        ov_cond = np.zeros(cfg.OV_S, np.float32)
        ov_src = np.full(cfg.OV_S, SENT, np.int32)
        ovslot = ov_row_of[n[m2]] * cfg.OV_PAD + (rank[m2] - MAXPAD)
        ov_cond[ovslot] = c[m2]
        ov_src[ovslot] = enc1(pos_of[s[m2]])
        ov_node_g = np.full(cfg.OV_ROWS, SENT, np.int32)
        ov_node_g[: len(ov_nodes)] = enc1(pos_of[base + ov_nodes])
        ov_out = np.full(cfg.OV_ROWS, 128 * REFF - 1, np.int32)
        ov_out[: len(ov_nodes)] = rowmap[ov_nodes].astype(np.int32)
        ovsp = ov_src.reshape(cfg.OV_PARTS, cfg.OV_SPP)
        ovs_cols = cfg.OV_S // 128 // cfg.OV_PARTS
        ov_srct = np.concatenate(
            [_wrap_cols(cfg, ovsp[q], ovs_cols) for q in range(cfg.OV_PARTS)], axis=1
        )
        ov_nodet = _wrap_cols(cfg, ov_node_g, cfg.OV_ROWS // 128)
        ov_outt = _wrap_cols(cfg, ov_out, cfg.OV_ROWS // 128)

        Tl = np.zeros(128 * REFF, np.float32)
        Tl[rowmap] = T[base : base + cfg.NODES_PC]

        in_maps.append(
            {
                "T_full": padded_perm(T),
                "L_full": padded_perm(L),
                "D_full": padded_perm(D),
                "F_full": padded_perm(avg_f),
                "T_loc": Tl.reshape(128, REFF),
                "ts": ts,
                "cond1": condA.reshape(128, 2 * UNITS_P),
                "src1t": src1t,
                "ov_cond": ov_cond.reshape(cfg.OV_PARTS, cfg.OV_SPP),
                "ov_srct": ov_srct,
                "ov_nodet": ov_nodet,
                "ov_outt": ov_outt,
            }
        )
    return in_maps


def host_prep(cfg: Cfg, T, L, D, avg_f, conductivity, src, tgt, time_step):
    if cfg.pair_units:
        return host_prep_pairs(
            cfg, T, L, D, avg_f, conductivity, src, tgt, time_step
        )
    T = np.asarray(T, np.float32)
    L = np.asarray(L, np.float32)
    D = np.asarray(D, np.float32)
    avg_f = np.asarray(avg_f, np.float32)
    cond = np.asarray(conductivity, np.float32)
    src = np.asarray(src, np.int32)
    tgt = np.asarray(tgt, np.int32)
    ts = np.full((128, 1), np.float32(np.asarray(time_step)), np.float32)

    def padded(x):
        p = np.zeros(cfg.NT_PAD, np.float32)
        p[: cfg.N_NODES] = x
        return p.reshape(128, cfg.NT_PART)

    Tp, Lp, Dp, Fp = padded(T), padded(L), padded(D), padded(avg_f)

    order = np.argsort(tgt, kind="stable")
    tgt_s = tgt[order]
    src_s = src[order]
    cond_s = cond[order]
    SENT = _sentinel(cfg)

    pads = np.array([p for p, _ in cfg.tiers], np.int64)
    caps = np.array([r * 128 for _, r in cfg.tiers], np.int64)
    row_off = np.concatenate([[0], np.cumsum([r for _, r in cfg.tiers])])
    slot_off = np.concatenate([[0], np.cumsum([p * r for p, r in cfg.tiers])])
    NC = len(cfg.tiers)
    REFF = cfg.R_PART_EFF

    in_maps = []
    for k in range(N_CORES):
        base = k * cfg.NODES_PC
        lo, hi = np.searchsorted(tgt_s, [base, base + cfg.NODES_PC])
        n = (tgt_s[lo:hi] - base).astype(np.int64)
        s = src_s[lo:hi]
        c = cond_s[lo:hi]

        deg = np.bincount(n, minlength=cfg.NODES_PC)
        starts = np.concatenate([[0], np.cumsum(deg)[:-1]])
        rank = np.arange(len(n), dtype=np.int64) - starts[n]

        # tier assignment with cascade when a tier overflows its capacity
        cls = np.searchsorted(pads, np.minimum(deg, cfg.MAXPAD))
        for ci in range(NC - 1):
            idx = np.flatnonzero(cls == ci)
            if len(idx) > caps[ci]:
                cls[idx[caps[ci] :]] = ci + 1
        assert (cls == NC - 1).sum() <= caps[NC - 1], "top tier overflow"

        rowp = np.empty(cfg.NODES_PC, np.int64)
        rowr = np.empty(cfg.NODES_PC, np.int64)
        sbase = np.empty(cfg.NODES_PC, np.int64)
        padv = np.empty(cfg.NODES_PC, np.int64)
        for ci in range(NC):
            idx = np.flatnonzero(cls == ci)
            i = np.arange(len(idx))
            rowp[idx] = i % 128
            rowr[idx] = row_off[ci] + i // 128
            sbase[idx] = slot_off[ci] + (i // 128) * pads[ci]
            padv[idx] = pads[ci]

        pad1_eff = padv[n]
        m1 = rank < pad1_eff
        cond1 = np.zeros(128 * cfg.SLOTS_P, np.float32)
        src1 = np.full(128 * cfg.SLOTS_P, SENT, np.int32)
        slot = rowp[n] * cfg.SLOTS_P + sbase[n] + rank
        cond1[slot[m1]] = c[m1]
        src1[slot[m1]] = _encode_idx(cfg, s[m1].astype(np.int64))
        rowmap = rowp * REFF + rowr  # node -> out flat index
        _ROWMAPS[k] = rowmap
        # per-partition idx tiles, column-major wrapped
        src1_p = src1.reshape(128, cfg.SLOTS_P)
        src1t = np.concatenate(
            [_wrap_cols(cfg, src1_p[p], cfg.IDX_C) for p in range(128)], axis=1
        )

        m2 = ~m1  # only deg > MAXPAD rows reach here (rank >= MAXPAD)
        ov_nodes = np.unique(n[m2])
        assert len(ov_nodes) <= cfg.OV_ROWS, (len(ov_nodes), cfg.OV_ROWS)
        assert deg.max() <= cfg.MAXPAD + cfg.OV_PAD, deg.max()
        ov_row_of = np.full(cfg.NODES_PC, -1, np.int64)
        ov_row_of[ov_nodes] = np.arange(len(ov_nodes))
        ov_cond = np.zeros(cfg.OV_S, np.float32)
        ov_src = np.full(cfg.OV_S, SENT, np.int32)
        ovslot = ov_row_of[n[m2]] * cfg.OV_PAD + (rank[m2] - cfg.MAXPAD)
        ov_cond[ovslot] = c[m2]
        ov_src[ovslot] = _encode_idx(cfg, s[m2].astype(np.int64))
        ov_node_g = np.full(cfg.OV_ROWS, SENT, np.int32)
        ov_node_g[: len(ov_nodes)] = _encode_idx(cfg, (base + ov_nodes).astype(np.int64))
        ov_out = np.full(cfg.OV_ROWS, 128 * REFF - 1, np.int32)
        ov_out[: len(ov_nodes)] = rowmap[ov_nodes].astype(np.int32)
        # overflow slot pairs: OV_PARTS per-partition instructions
        ovsp = ov_src.reshape(cfg.OV_PARTS, cfg.OV_SPP)
        ovs_cols = cfg.OV_S // 128 // cfg.OV_PARTS
        ov_srct = np.concatenate(
            [_wrap_cols(cfg, ovsp[q], ovs_cols) for q in range(cfg.OV_PARTS)], axis=1
        )
        ov_nodet = _wrap_cols(cfg, ov_node_g, cfg.OV_ROWS // 128)
        ov_outt = _wrap_cols(cfg, ov_out, cfg.OV_ROWS // 128)

        Tl = np.zeros(128 * REFF, np.float32)
        Tl[rowmap] = T[base : base + cfg.NODES_PC]

        in_maps.append(
            {
                "T_full": Tp,
                "L_full": Lp,
                "D_full": Dp,
                "F_full": Fp,
                "T_loc": Tl.reshape(128, REFF),
                "ts": ts,
                "cond1": cond1.reshape(128, cfg.SLOTS_P),
                "src1t": src1t,
                "ov_cond": ov_cond.reshape(cfg.OV_PARTS, cfg.OV_SPP),
                "ov_srct": ov_srct,
                "ov_nodet": ov_nodet,
                "ov_outt": ov_outt,
            }
        )
    return in_maps


def unshard(cfg: Cfg, results):
    outs = []
    for k in range(N_CORES):
        o = np.asarray(results[k]["out"], np.float32).reshape(128 * cfg.R_PART_EFF)
        outs.append(o[_ROWMAPS[k]])
    return np.concatenate(outs)


# --------------------------------------------------------------------------
# entry point
# --------------------------------------------------------------------------
_NC_CACHE = {}


def _get_program(cfg: Cfg):
    if cfg not in _NC_CACHE:
        _NC_CACHE[cfg] = build_program(cfg)
    return _NC_CACHE[cfg]


def kernel(**inputs) -> np.ndarray:
    from concourse.bass_utils import run_bass_kernel_spmd

    cfg = FULL_P
    nc = _get_program(cfg)
    in_maps = host_prep(cfg, **inputs)
    res = run_bass_kernel_spmd(nc, in_maps, core_ids=list(range(N_CORES)))
    return unshard(cfg, res.results)
